# revision 1
# baseline (speedup 1.0000x reference)
"""Trainium2 Bass kernel for nn_DirectionVarEntropy.

Computes, per 14x14 patch and channel:
  - pixel-value entropy (256-bin histogram of round(x*255))
  - direction variance psi of 3x3-DCT sliding-window directional stds
  - richness = mean_c(psi_m * entropy)  ->  output (B, Hp, Wp)

Sharding: pure data parallel over batch, 2 images per core on 8 cores.

Per-core layout: 2048 spatial patches x 3 channels = 6144 patch-channels,
mapped to [128 partitions x 48 free segments]; seg s = t*3 + c where
t = spatial_patch // 128, partition p = spatial_patch % 128.

Entropy (the histogram_binning part): instead of materializing 256-bin
histograms (which needs either scatter-add hardware this chip lacks, or
256 compare+reduce passes dominated by per-instruction overhead), compute
per-pixel own-bin counts c_p = #\{q: pi_q == pi_p\} with 195 circular-shift
tensor_tensor(is_equal) + add passes in bf16 (DVE 2x mode), each one
instruction covering all 48 segments.  Then
  E = log2(196) - mean_p ln(c_p)/ln 2
which equals the dense-histogram entropy up to the reference's 1e-10
epsilon terms (~1e-6 relative).  Shifted reads stay 4B-aligned via two
doubled pixel buffers (one rotated by a pixel) so the DVE keeps its fast
mode for odd shifts.

DCT part: explicit 9 coefficient planes via separable 3-tap convolutions
(tensor_scalar + scalar_tensor_tensor on shifted access patterns), group
sums / stds / psi in fp32 on DVE; ACT does squares and sqrt via
exp(0.5*ln x) so every activation stays in one LUT function-set (no
1.3us table reloads).  SBUF is phase-scoped: conv/psi blocks run first
(X + work pool), then the entropy phase reuses that space.
"""

import functools

import numpy as np

import concourse.bacc as bacc
import concourse.bass as bass
import concourse.mybir as mybir
from concourse import bass_utils
from concourse.tile import TileContext

P = 128
PH = 14
NWIN = 12          # sliding 3x3 positions per axis
NPIX = PH * PH     # 196
BINS = 256
LN2 = 0.6931471805599453
F32 = mybir.dt.float32
BF16 = mybir.dt.bfloat16
ALU = mybir.AluOpType
ACTF = mybir.ActivationFunctionType

# problem shape (hardcoded per contract)
B_FULL, C, H, W = 16, 3, 448, 448
N_CORES = 8
B_CORE = B_FULL // N_CORES      # 2
HP = H // PH                    # 32
T_BLKS = B_CORE * HP * HP // P  # 16 t-blocks of 128 spatial patches
SEGS = T_BLKS * C               # 48


def _build(dct_flat: tuple, segs: int = SEGS, bins: int = BINS,
           nb: int = 3, act_bins: int = 0) -> bass.Bass:
    """Build the SPMD single-core program. dct_flat: 9 floats, row major."""
    D = np.asarray(dct_flat, np.float64).reshape(3, 3)
    nc = bacc.Bacc("TRN2", debug=False, enable_asserts=False)

    x_d = nc.dram_tensor("x", (B_CORE, C, H, W), F32, kind="ExternalInput")
    out_d = nc.dram_tensor("out", (B_CORE, HP, HP), F32, kind="ExternalOutput")
    # (b, c, hp, i, wp, j) view of DRAM input, reordered to (b c hp wp i j)
    xv = x_d.ap().rearrange("b c (hp i) (wp j) -> b c hp wp i j", i=PH, j=PH)
    ov = out_d.ap()

    n_blocks = (segs + nb - 1) // nb

    with TileContext(nc) as tc:
        with tc.tile_pool(name="persist", bufs=1) as pp:
            X = pp.tile([P, segs, PH, PH], F32)
            Xf = X.rearrange("p s i j -> p (s i j)")
            TMP = pp.tile([P, (segs // 8) * NPIX], F32)
            dummy = pp.tile([P, NPIX], BF16)
            pdum = pp.tile([P, NWIN * NWIN], F32)
            psi_acc = pp.tile([P, segs], F32)
            e_acc = pp.tile([P, segs], F32)
            rich = pp.tile([P, segs], F32)
            rich3 = rich.rearrange("p (t c) -> p t c", c=C)
            tsum = pp.tile([P, segs // C], F32)
            osb = pp.tile([P, segs // C], F32)

            # ---- input DMAs: per (t, c, p1) a [32, 14, 14] strided load ----
            for t in range(T_BLKS):
                b = t // (T_BLKS // B_CORE)
                hp0 = (t % (T_BLKS // B_CORE)) * 4
                for c in range(C):
                    s = t * C + c
                    for p1 in range(4):
                        nc.sync.dma_start(
                            X[p1 * 32:(p1 + 1) * 32, s],
                            xv[b, c, hp0 + p1],
                        )
            # Per-DMA same-engine absorber copies: each waits on exactly one
            # DMA queue semaphore; all downstream DVE reads of X then order
            # behind these in program order (no multi-sem waits, which
            # overflow the ISA sync-wait slots).
            for t in range(T_BLKS):
                for c in range(C):
                    s = t * C + c
                    for p1 in range(4):
                        sl = X[p1 * 32:(p1 + 1) * 32, s]
                        nc.vector.tensor_copy(sl, sl)

            d = [[float(D[r, c]) for c in range(3)] for r in range(3)]

            wp_ctx = tc.tile_pool(name="work", bufs=2)
            wp = wp_ctx.__enter__()
            for blk in range(n_blocks):
                s0 = blk * nb
                sn = min(nb, segs - s0)
                # conv tiles for this block
                V = [wp.tile([P, nb, NWIN, PH], F32, tag=f"V{r}", name=f"V{r}")
                     for r in range(3)]
                Y = [[wp.tile([P, nb, NWIN, NWIN], F32, tag=f"Y{r}{c}", name=f"Y{r}{c}")
                      for c in range(3)] for r in range(3)]
                xb = X[:, s0:s0 + sn]

                # vertical convs V_r(i,j) = sum_k D[r,k] x(i+k, j)
                for r in range(3):
                    vb = V[r][:, :sn]
                    nc.vector.tensor_scalar(
                        vb, xb[:, :, 0:NWIN, :], d[r][0], None, ALU.mult)
                    for k in (1, 2):
                        nc.vector.scalar_tensor_tensor(
                            vb, xb[:, :, k:k + NWIN, :], d[r][k], vb,
                            ALU.mult, ALU.add)
                # horizontal convs Y_rc(i,j) = sum_l D[c,l] V_r(i, j+l)
                for r in range(3):
                    vb = V[r][:, :sn]
                    for c in range(3):
                        yb = Y[r][c][:, :sn]
                        nc.vector.tensor_scalar(
                            yb, vb[:, :, :, 0:NWIN], d[c][0], None, ALU.mult)
                        for l in (1, 2):
                            nc.vector.scalar_tensor_tensor(
                                yb, vb[:, :, :, l:l + NWIN], d[c][l], yb,
                                ALU.mult, ALU.add)

                # group sums of Y (pre-square): rows, cols, diag, anti-diag
                GROUPS = (
                    [[(r, 0), (r, 1), (r, 2)] for r in range(3)]       # rows
                    + [[(0, c), (1, c), (2, c)] for c in range(3)]     # cols
                    + [[(0, 0), (1, 1), (2, 2)],                       # diag
                       [(0, 2), (1, 1), (2, 0)]]                       # anti
                )
                M = [wp.tile([P, nb, NWIN, NWIN], F32, tag=f"M{g}", name=f"M{g}")
                     for g in range(8)]
                SS = [wp.tile([P, nb, NWIN, NWIN], F32, tag=f"SS{g}", name=f"SS{g}")
                      for g in range(8)]
                for g, mem in enumerate(GROUPS):
                    mb = M[g][:, :sn]
                    (r0, c0), (r1, c1), (r2, c2) = mem
                    nc.vector.tensor_add(
                        mb, Y[r0][c0][:, :sn], Y[r1][c1][:, :sn])
                    nc.vector.tensor_add(mb, mb, Y[r2][c2][:, :sn])
                    # Msq = (M/3)^2 in place
                    nc.scalar.activation(mb, mb, ACTF.Square, scale=1.0 / 3)
                # squares of Y in place
                for r in range(3):
                    for c in range(3):
                        yb = Y[r][c][:, :sn]
                        nc.scalar.activation(yb, yb, ACTF.Square)
                for g, mem in enumerate(GROUPS):
                    sb = SS[g][:, :sn]
                    (r0, c0), (r1, c1), (r2, c2) = mem
                    nc.vector.tensor_add(
                        sb, Y[r0][c0][:, :sn], Y[r1][c1][:, :sn])
                    nc.vector.tensor_add(sb, sb, Y[r2][c2][:, :sn])
                    # std^2 = SS/3 - (M/3)^2, clamp, sqrt -> sigma in SS tile
                    # (sqrt via exp(0.5*ln x): keeps every ACT func in the
                    # natural_log_exp_and_others table set -- no table swaps)
                    nc.vector.scalar_tensor_tensor(
                        sb, sb, 1.0 / 3, M[g][:, :sn], ALU.mult, ALU.subtract)
                    nc.vector.tensor_scalar_max(sb, sb, 1e-38)
                    nc.scalar.activation(sb, sb, ACTF.Ln)
                    nc.scalar.activation(sb, sb, ACTF.Exp, scale=0.5)

                U1 = wp.tile([P, nb, NWIN, NWIN], F32, tag="U1", name="U1")
                U2 = wp.tile([P, nb, NWIN, NWIN], F32, tag="U2", name="U2")
                t1 = wp.tile([P, nb, NWIN, NWIN], F32, tag="t1", name="t1")
                t2 = wp.tile([P, nb, NWIN, NWIN], F32, tag="t2", name="t2")
                A = wp.tile([P, nb, NWIN, NWIN], F32, tag="A", name="A")
                sum2 = wp.tile([P, nb, NWIN, NWIN], F32, tag="sum2", name="sum2")
                aq = wp.tile([P, nb, NWIN, NWIN], F32, tag="aq", name="aq")
                s_t = wp.tile([P, nb, NWIN, NWIN], F32, tag="s_t", name="s_t")
                ssq = wp.tile([P, nb, NWIN, NWIN], F32, tag="ssq", name="ssq")
                rinv = wp.tile([P, nb, NWIN, NWIN], F32, tag="rinv", name="rinv")
                psi = wp.tile([P, nb, NWIN, NWIN], F32, tag="psi", name="psi")
                u1, u2 = U1[:, :sn], U2[:, :sn]
                tb1, tb2 = t1[:, :sn], t2[:, :sn]
                ab = A[:, :sn]
                s2b, aqb = sum2[:, :sn], aq[:, :sn]
                stb, ssqb, rb, psib = (s_t[:, :sn], ssq[:, :sn],
                                       rinv[:, :sn], psi[:, :sn])
                sig = [SS[g][:, :sn] for g in range(8)]

                nc.vector.tensor_add(u1, sig[0], sig[1])
                nc.vector.tensor_add(u1, u1, sig[2])
                nc.vector.tensor_add(u2, sig[3], sig[4])
                nc.vector.tensor_add(u2, u2, sig[5])
                # A = U1/3 + U2/3 + sig6 + sig7
                nc.vector.scalar_tensor_tensor(
                    tb1, u1, 1.0 / 3, sig[6], ALU.mult, ALU.add)
                nc.vector.scalar_tensor_tensor(
                    tb2, u2, 1.0 / 3, sig[7], ALU.mult, ALU.add)
                nc.vector.tensor_add(ab, tb1, tb2)
                # sum of squared directional stds
                nc.scalar.activation(u1, u1, ACTF.Square, scale=1.0 / 3)
                nc.scalar.activation(u2, u2, ACTF.Square, scale=1.0 / 3)
                nc.scalar.activation(sig[6], sig[6], ACTF.Square)
                nc.scalar.activation(sig[7], sig[7], ACTF.Square)
                nc.vector.tensor_add(tb1, u1, u2)
                nc.vector.tensor_add(tb2, sig[6], sig[7])
                nc.vector.tensor_add(s2b, tb1, tb2)
                # psi = (sum2 - A^2/4) / (3 * (A/4 + 1e-8)^2)
                nc.scalar.activation(aqb, ab, ACTF.Square, scale=0.5)
                nc.vector.tensor_sub(s2b, s2b, aqb)
                nc.vector.tensor_scalar(
                    stb, ab, 0.25, 1e-8, ALU.mult, ALU.add)
                nc.scalar.activation(ssqb, stb, ACTF.Square)
                nc.vector.reciprocal(rb, ssqb)
                nc.vector.scalar_tensor_tensor(
                    psib, s2b, 1.0 / 3, rb, ALU.mult, ALU.mult)
                # psi_m accumulate per seg
                for i in range(sn):
                    s = s0 + i
                    nc.vector.tensor_scalar(
                        pdum, psib[:, i].rearrange("p i j -> p (i j)"),
                        1.0, None, ALU.mult, ALU.add,
                        accum_out=psi_acc[:, s:s + 1])

            wp_ctx.__exit__(None, None, None)
            ep_ctx = tc.tile_pool(name="ent", bufs=1)
            ep = ep_ctx.__enter__()
            # ---- quantize: pi = round(x*255) via the 2^23 RNE trick ----
            # PI2: per seg the 196 pixel codes stored twice (j and j+196) so
            # circularly shifted reads stay within the seg row.  PI2o: the
            # same, rotated by one pixel, so odd shifts read at even (4B)
            # offsets and keep the DVE 2x mode.
            PI2 = ep.tile([P, segs, 2 * NPIX], BF16)
            PI2o = ep.tile([P, segs, 2 * NPIX], BF16)
            TWO23 = float(2 ** 23)
            qch = (segs // 8) * NPIX
            TMP3 = TMP.rearrange("p (s k) -> p s k", k=NPIX)
            spq = segs // 8
            for q in range(8):
                nc.vector.tensor_scalar(
                    TMP, Xf[:, q * qch:(q + 1) * qch], 255.0, TWO23,
                    ALU.mult, ALU.add)
                nc.vector.tensor_scalar(
                    PI2[:, q * spq:(q + 1) * spq, 0:NPIX], TMP3, TWO23,
                    None, ALU.subtract)
            nc.vector.tensor_copy(PI2[:, :, NPIX:2 * NPIX],
                                  PI2[:, :, 0:NPIX])
            nc.vector.tensor_copy(PI2o[:, :, 0:2 * NPIX - 1],
                                  PI2[:, :, 1:2 * NPIX])
            nc.vector.tensor_copy(PI2o[:, :, 2 * NPIX - 1:2 * NPIX],
                                  PI2[:, :, 1:2])

            # ---- entropy: per-pixel own-bin counts via 195 shifted
            # equality passes (all segs per instruction), then
            # E = log2(N) - mean_p ln(count_p) / ln 2 ----
            ACC = ep.tile([P, segs, NPIX], BF16)
            EQT = ep.tile([P, segs, NPIX], BF16)
            base = PI2[:, :, 0:NPIX]
            ACCf = ACC.rearrange("p s k -> p (s k)")
            EQTf = EQT.rearrange("p s k -> p (s k)")
            nc.vector.tensor_tensor(ACC, base, PI2o[:, :, 0:NPIX],
                                    ALU.is_equal)
            for s in range(2, NPIX):
                if s % 2 == 0:
                    shifted = PI2[:, :, s:s + NPIX]
                else:
                    shifted = PI2o[:, :, s - 1:s - 1 + NPIX]
                nc.vector.tensor_tensor(EQT, base, shifted, ALU.is_equal)
                nc.vector.tensor_tensor(ACC, ACC, EQT, ALU.add)
            # ln(count) with the +1 self-match folded into the ACT bias
            LNP = ep.tile([P, segs, NPIX], F32)
            LNPf = LNP.rearrange("p s k -> p (s k)")
            nc.scalar.activation(LNPf, ACCf, ACTF.Ln, bias=1.0)
            for s in range(segs):
                nc.vector.tensor_scalar(
                    dummy, LNP[:, s], 1.0, None, ALU.mult,
                    ALU.add, accum_out=e_acc[:, s:s + 1])

            ep_ctx.__exit__(None, None, None)
            # ---- richness = psi_m * entropy, mean over channels ----
            import math
            nc.vector.tensor_scalar(
                e_acc, e_acc, -1.0 / (NPIX * LN2), float(math.log2(NPIX)),
                ALU.mult, ALU.add)
            nc.vector.scalar_tensor_tensor(
                rich, psi_acc, 1.0 / (NWIN * NWIN), e_acc,
                ALU.mult, ALU.mult)
            nc.vector.tensor_add(tsum, rich3[:, :, 0], rich3[:, :, 1])
            nc.vector.tensor_add(tsum, tsum, rich3[:, :, 2])
            nc.vector.tensor_scalar(osb, tsum, 1.0 / C, None, ALU.mult)

            # ---- output DMAs ----
            for t in range(T_BLKS):
                b = t // (T_BLKS // B_CORE)
                hp0 = (t % (T_BLKS // B_CORE)) * 4
                nc.sync.dma_start(ov[b, hp0:hp0 + 4], osb[:, t:t + 1])

    nc.compile()
    return nc


@functools.lru_cache(maxsize=4)
def _build_cached(dct_flat: tuple) -> bass.Bass:
    return _build(dct_flat)


def kernel(x, dct_matrix):
    x = np.ascontiguousarray(np.asarray(x, dtype=np.float32))
    D = np.asarray(dct_matrix, dtype=np.float32)
    assert x.shape == (B_FULL, C, H, W), x.shape
    nc = _build_cached(tuple(float(v) for v in D.flatten()))
    in_maps = [
        {"x": np.ascontiguousarray(x[i * B_CORE:(i + 1) * B_CORE])}
        for i in range(N_CORES)
    ]
    res = bass_utils.run_bass_kernel_spmd(
        nc, in_maps, core_ids=list(range(N_CORES)))
    out = np.concatenate([r["out"] for r in res.results], axis=0)
    return out.astype(np.float32)



# revision 19
# speedup vs baseline: 5.5662x; 5.5662x over previous
"""Trainium2 Bass kernel for nn_DirectionVarEntropy (v2).

Per 14x14 patch and channel:
  - pixel-value entropy (256-bin histogram of round(x*255))
  - direction variance psi of 3x3-DCT sliding-window directional stds
  - richness = mean_c(psi_m * entropy)  ->  output (B, Hp, Wp)

Sharding: pure data parallel over batch, 2 images per core on 8 cores.

v2 design (vs the O(N^2) shifted-equality v1):

Entropy: split each 8-bit code pi into hi = pi>>4 and lo = pi&15.  The
256-bin histogram of a patch is H = U^T V where U/V are the 196x16
one-hots of 16*hi/lo -- TensorEngine matmuls with pixels as the
contraction dim, batched 8 patches per 128x128 matmul (diagonal 16x16
blocks are the per-patch histograms; off-diagonal blocks are cross-
patch garbage that is masked out later).  Sum_b c ln c is recovered
with one Ln pass (ACT), a c*ln(c) multiply, an in-column tree
reduction over the 16 v-bins (DVE 4x), a block-diag-ones matmul (sums
over the 16 u-bins), and a masked ones-matmul that extracts the
per-patch diagonal of the remaining 8x8 blocks.  Codes reach
pixel-major layout via 4 xbar dma-transposes, whose semantics are
out[q, s, p] = in[p, s*128 + q].

Conv/psi: separable 3-tap convs to the 9 DCT planes in fp16, with all
elementwise work expressed as tensor_scalar / scalar_tensor_tensor
(DVE 4x mode on packed 2-byte dtypes).  Directional variances use the
pairwise-difference identity var(a,b,c) = ((a-b)^2+(b-c)^2+(a-c)^2)/9
(cancellation-free, no clamps needed).  ACT uses only {Identity,
Square->none, Sqrt} during conv and {Identity, Copy, Ln} for entropy,
so activation-table loads are O(1) instead of per-window.
"""

import functools
import math

import numpy as np

import concourse.bacc as bacc
import concourse.bass as bass
import concourse.mybir as mybir
from concourse import bass_utils
from concourse.tile import TileContext

P = 128
PH = 14
NWIN = 12          # sliding 3x3 positions per axis
NPIX = PH * PH     # 196
LN2 = 0.6931471805599453
F32 = mybir.dt.float32
F16 = mybir.dt.float16
ALU = mybir.AluOpType
ACTF = mybir.ActivationFunctionType
TWO23 = float(2 ** 23)
PAD_HI = 63744.0   # fp16-exact, != 16*u for all u in 0..15
MAGIC = float(3 * 2 ** 22)   # 1.5*2^23: integer-ulp zone covers negatives

# problem shape (hardcoded per contract)
B_FULL, C, H, W = 16, 3, 448, 448
N_CORES = 8
B_CORE = B_FULL // N_CORES      # 2
HP = H // PH                    # 32
T_BLKS = B_CORE * HP * HP // P  # 16 t-blocks of 128 spatial patches
SEGS = T_BLKS * C               # 48
NPAT = SEGS * P                 # 6144 patch-channels per core


def _stt(nc, out, in0, scalar, in1, op0, op1):
    nc.vector.scalar_tensor_tensor(out, in0, scalar, in1, op0, op1)


def _build(dct_flat: tuple, nb: int = 4, ch_pat: int = 768,
           debug_out: str = "rich") -> bass.Bass:
    """Build the SPMD single-core program. dct_flat: 9 floats, row major."""
    D = np.asarray(dct_flat, np.float64).reshape(3, 3)
    nc = bacc.Bacc("TRN2", debug=False, enable_asserts=False)

    x_d = nc.dram_tensor("x", (B_CORE, C, H, W), F32, kind="ExternalInput")
    out_d = nc.dram_tensor("out", (B_CORE, HP, HP), F32, kind="ExternalOutput")
    xv = x_d.ap().rearrange("b c (hp i) (wp j) -> b c hp wp i j", i=PH, j=PH)
    ov = out_d.ap()

    d = [[float(D[r, c]) for c in range(3)] for r in range(3)]
    n_blocks = SEGS // nb
    n_chunks = NPAT // ch_pat          # one-hot build chunks
    grp_per_ch = ch_pat // 8           # histogram groups per chunk
    n_batch = ch_pat // 64             # 8-group batches per chunk

    with TileContext(nc) as tc:
        with tc.tile_pool(name="persist", bufs=1) as pp:
            psi_acc = pp.tile([P, SEGS], F32)
            ent_pm = pp.tile([P, SEGS], F32)
            pdum = pp.tile([P, NWIN * NWIN], F16)
            rich = pp.tile([P, SEGS], F32)
            rich3 = rich.rearrange("p (t c) -> p t c", c=C)
            tsum = pp.tile([P, SEGS // C], F32)
            osb = pp.tile([P, SEGS // C], F32)
            # pixel-major code tiles: [q', (s, p)] for pixel q = h*128+q'
            xphi = [pp.tile([P, SEGS, P], F16, name=f"xphi{h}") for h in (0, 1)]
            xplo = [pp.tile([P, SEGS, P], F16, name=f"xplo{h}") for h in (0, 1)]
            xphi_f = [t.rearrange("q s p -> q (s p)") for t in xphi]
            xplo_f = [t.rearrange("q s p -> q (s p)") for t in xplo]
            # X16 for the conv phase
            X16 = pp.tile([P, SEGS, PH, PH], F16)
            # constants for the entropy tail
            bd = pp.tile([P, 8], F16)        # bd[p, t] = (p % 8 == t)
            mask64 = pp.tile([8, 64], F16)   # mask[t, g*8+r] = (t == r)
            ones8 = pp.tile([8, 1], F32)
            bias_nm16 = pp.tile([P, 1], F32)  # -16*MAGIC
            bias_n223 = pp.tile([P, 1], F32)  # -2^23
            bias_eps = pp.tile([P, 1], F32)   # 1e-30

            # ---- constants via memsets ----
            nc.vector.memset(bd, 0.0)
            nc.vector.memset(mask64, 0.0)
            nc.vector.memset(ones8, 1.0)
            nc.vector.memset(bias_nm16, -16.0 * MAGIC)
            nc.vector.memset(bias_n223, -TWO23)
            nc.vector.memset(bias_eps, 1e-30)
            # mask64[t, g*8+r] = (t == r) via iota + per-partition compare
            ridx_i = pp.tile([8, 64], mybir.dt.int32)
            ridx_f = pp.tile([8, 64], F32)
            tidx_i = pp.tile([8, 1], mybir.dt.int32)
            tidx_f = pp.tile([8, 1], F32)
            nc.gpsimd.iota(ridx_i.rearrange("p (a b) -> p a b", b=8),
                           [[0, 8], [1, 8]], base=0, channel_multiplier=0)
            nc.gpsimd.iota(tidx_i, [[0, 1]], base=0, channel_multiplier=1)
            nc.vector.tensor_copy(ridx_f, ridx_i)
            nc.vector.tensor_copy(tidx_f, tidx_i)
            nc.vector.tensor_scalar(
                mask64, ridx_f, tidx_f, None, ALU.is_equal)
            # bd[p, t] = (p % 8 == t) via partition iota + exact mod-8
            pidx_i = pp.tile([P, 1], mybir.dt.int32)
            pidx_f = pp.tile([P, 1], F32)
            pf1 = pp.tile([P, 1], F32)
            pm8 = pp.tile([P, 1], F32)
            nc.gpsimd.iota(pidx_i, [[0, 1]], base=0, channel_multiplier=1)
            nc.vector.tensor_copy(pidx_f, pidx_i)
            nc.vector.tensor_scalar(
                pf1, pidx_f, 0.125, -0.4375, ALU.mult, ALU.add)
            nc.vector.tensor_scalar(
                pf1, pf1, MAGIC, None, ALU.add)
            nc.vector.tensor_scalar(
                pf1, pf1, MAGIC, 8.0, ALU.subtract, ALU.mult)
            _stt(nc, pm8, pf1, -1.0, pidx_f, ALU.mult, ALU.add)
            for t in range(8):
                nc.vector.tensor_scalar(
                    bd[:, t:t + 1], pm8, float(t), None, ALU.is_equal)

            # =========== input DMAs (patch-major) + fp16 absorb ===========
            lp_ctx = tc.tile_pool(name="load", bufs=1)
            lp = lp_ctx.__enter__()
            X32 = lp.tile([P, SEGS, NPIX], F32)
            X32v = X32.rearrange("p s (i j) -> p s i j", j=PH)
            for t in range(T_BLKS):
                b = t // (T_BLKS // B_CORE)
                hp0 = (t % (T_BLKS // B_CORE)) * 4
                for c in range(C):
                    s = t * C + c
                    for p1 in range(4):
                        nc.sync.dma_start(
                            X32v[p1 * 32:(p1 + 1) * 32, s],
                            xv[b, c, hp0 + p1],
                        )
            # absorber + fp16 convert, one per seg: waits on <=4 DMA sems;
            # all later DVE reads of X32 order behind these in program order.
            for s in range(SEGS):
                nc.vector.tensor_scalar(
                    X16[:, s].rearrange("p i j -> p (i j)"),
                    X32[:, s], 1.0, None, ALU.mult)

            # =========== quantize to hi/lo codes (patch-major) ============
            # h-layout tiles [p, h, s, q'] so that [:, h] flattens to free
            # index (s*128 + q') -- the exact xbar-transpose input layout.
            qp_ctx = tc.tile_pool(name="quant", bufs=1)
            qp = qp_ctx.__enter__()
            HI16 = qp.tile([P, 2, SEGS, P], F16)
            LOc = qp.tile([P, 2, SEGS, P], F16)
            CHC = float(2 ** 23 - 2 ** 19) - 15.0 / 32.0
            SH = SEGS // 2
            for h2 in range(2):
                q0 = h2 * P
                qn = min(P, NPIX - q0)
                for sh in range(2):
                    sl = slice(sh * SH, (sh + 1) * SH)
                    Th = qp.tile([P, SH, P], F32, tag="Th", name="Th")
                    H1h = qp.tile([P, SH, P], F32, tag="H1h", name="H1h")
                    xs = X32[:, sl, q0:q0 + qn]
                    # T = x*255 + 2^23  (exact RNE binning, f32 on DVE)
                    nc.vector.tensor_scalar(
                        Th[:, :, 0:qn], xs, 255.0, TWO23, ALU.mult, ALU.add)
                    # floor(pi/16) = round(pi/16 - 7/16) via the 1.5*2^23
                    # magic (integer ulp even for small/negative values);
                    # two exact steps (a fused constant is not f32-exact):
                    #   s1 = T/16 - (2^19 + 7/16) = pi/16 - 7/16   (exact)
                    #   H1 = s1 + MAGIC = MAGIC + floor(pi/16)     (RNE)
                    # s0 = T/16 - 2^19 = pi/16 (exact); then
                    # H1 = (s0 - 15/32) + MAGIC = MAGIC + floor(pi/16):
                    # (2m-15)/32 never hits a .5 tie for m in 0..15.
                    nc.vector.tensor_scalar(
                        H1h[:, :, 0:qn], Th[:, :, 0:qn], 1.0 / 16,
                        -float(2 ** 19), ALU.mult, ALU.add)
                    nc.vector.tensor_scalar(
                        H1h[:, :, 0:qn], H1h[:, :, 0:qn], 15.0 / 32.0, MAGIC,
                        ALU.subtract, ALU.add)
                    # HI16 = 16*floor(pi/16) = H1*16 - 16*MAGIC (fp16 exact)
                    nc.scalar.activation(
                        HI16[:, h2, sl, 0:qn], H1h[:, :, 0:qn],
                        ACTF.Identity, scale=16.0, bias=bias_nm16)
                    # LO = (T - 2^23) - HI16 = pi - 16*hi     (fp16 exact)
                    nc.vector.tensor_scalar(
                        LOc[:, h2, sl, 0:qn], Th[:, :, 0:qn], TWO23, None,
                        ALU.subtract)
                    _stt(nc, LOc[:, h2, sl, 0:qn], LOc[:, h2, sl, 0:qn], 1.0,
                         HI16[:, h2, sl, 0:qn], ALU.mult, ALU.subtract)
            # pad pixels (q >= 196) never match any hi bin
            nc.vector.memset(HI16[:, 1, :, NPIX - P:P], PAD_HI)
            nc.vector.memset(LOc[:, 1, :, NPIX - P:P], PAD_HI)

            # =========== xbar transposes to pixel-major ===================
            for h2 in range(2):
                nc.sync.dma_start(
                    xphi[h2], HI16[:, h2].rearrange("p s q -> p (s q)"),
                    transpose=True)
                nc.sync.dma_start(
                    xplo[h2], LOc[:, h2].rearrange("p s q -> p (s q)"),
                    transpose=True)
            qp_ctx.__exit__(None, None, None)
            lp_ctx.__exit__(None, None, None)

            # ================= conv / psi phase (fp16, DVE 4x) ============
            WDIM = [P, nb, NWIN, NWIN]
            wp_ctx = tc.tile_pool(name="work", bufs=2)
            wp = wp_ctx.__enter__()
            for blk in range(n_blocks):
                s0 = blk * nb
                xb = X16[:, s0:s0 + nb]
                V = [wp.tile([P, nb, NWIN, PH], F16, tag=f"V{r}",
                             name=f"V{r}") for r in range(3)]
                Y = [[wp.tile(WDIM, F16, tag=f"Y{r}{c}", name=f"Y{r}{c}")
                      for c in range(3)] for r in range(3)]
                # vertical convs V_r(i,j) = sum_k D[r,k] x(i+k, j)
                for r in range(3):
                    nc.vector.tensor_scalar(
                        V[r], xb[:, :, 0:NWIN, :], d[r][0], None, ALU.mult)
                    for k in (1, 2):
                        _stt(nc, V[r], xb[:, :, k:k + NWIN, :], d[r][k],
                             V[r], ALU.mult, ALU.add)
                # horizontal convs Y_rc(i,j) = sum_l D[c,l] V_r(i, j+l)
                for r in range(3):
                    for c in range(3):
                        yb = Y[r][c]
                        nc.vector.tensor_scalar(
                            yb, V[r][:, :, :, 0:NWIN], d[c][0], None,
                            ALU.mult)
                        for l in (1, 2):
                            _stt(nc, yb, V[r][:, :, :, l:l + NWIN], d[c][l],
                                 yb, ALU.mult, ALU.add)

                # sigma^2 per direction via pairwise differences:
                # var(a,b,c) = ((a-b)^2 + (b-c)^2 + (a-c)^2) / 9
                SIG2 = wp.tile([P, nb, 8, NWIN * NWIN], F16, tag="SIG2",
                               name="SIG2")
                SIG = wp.tile([P, nb, 8, NWIN * NWIN], F16, tag="SIG",
                              name="SIG")
                D1 = wp.tile(WDIM, F16, tag="D1", name="D1")
                D2 = wp.tile(WDIM, F16, tag="D2", name="D2")
                D3 = wp.tile(WDIM, F16, tag="D3", name="D3")
                GROUPS = (
                    [[(r, 0), (r, 1), (r, 2)] for r in range(3)]
                    + [[(0, c), (1, c), (2, c)] for c in range(3)]
                    + [[(0, 0), (1, 1), (2, 2)],
                       [(0, 2), (1, 1), (2, 0)]]
                )

                def wv(sl4):
                    # [P, nb, 144] view -> [P, nb, 12, 12]
                    return sl4.rearrange("p n (i j) -> p n i j", j=NWIN)

                for g, mem in enumerate(GROUPS):
                    a, b_, c_ = (Y[r][c] for (r, c) in mem)
                    _stt(nc, D1, a, 1.0, b_, ALU.mult, ALU.subtract)
                    _stt(nc, D2, b_, 1.0, c_, ALU.mult, ALU.subtract)
                    _stt(nc, D3, D1, 1.0, D2, ALU.mult, ALU.add)
                    _stt(nc, D1, D1, 1.0 / 9, D1, ALU.mult, ALU.mult)
                    _stt(nc, D2, D2, 1.0 / 9, D2, ALU.mult, ALU.mult)
                    _stt(nc, D3, D3, 1.0 / 9, D3, ALU.mult, ALU.mult)
                    _stt(nc, D1, D1, 1.0, D2, ALU.mult, ALU.add)
                    _stt(nc, wv(SIG2[:, :, g, :]), D1, 1.0, D3,
                         ALU.mult, ALU.add)
                # sigma = sqrt(sigma^2), all 8 dirs in one ACT pass
                nc.scalar.activation(
                    SIG.rearrange("p n g w -> p (n g w)"),
                    SIG2.rearrange("p n g w -> p (n g w)"), ACTF.Sqrt)

                # S-stats and psi via pairwise differences (ddof=1):
                # psi = sum_{i<j} (S_i - S_j)^2 / (12 * (A/4 + 1e-8)^2)
                U1 = wp.tile(WDIM, F16, tag="U1", name="U1")
                U2 = wp.tile(WDIM, F16, tag="U2", name="U2")
                A = wp.tile(WDIM, F16, tag="A", name="A")
                DP = wp.tile(WDIM, F16, tag="DP", name="DP")
                SUMP = wp.tile(WDIM, F16, tag="SUMP", name="SUMP")
                TA = wp.tile(WDIM, F16, tag="TA", name="TA")
                DEN = wp.tile(WDIM, F16, tag="DEN", name="DEN")
                RD = wp.tile(WDIM, F16, tag="RD", name="RD")
                PSI = wp.tile(WDIM, F16, tag="PSI", name="PSI")

                sgv = lambda g: wv(SIG[:, :, g, :])

                _stt(nc, U1, sgv(0), 1.0, sgv(1), ALU.mult, ALU.add)
                _stt(nc, U1, sgv(2), 1.0, U1, ALU.mult, ALU.add)
                _stt(nc, U2, sgv(3), 1.0, sgv(4), ALU.mult, ALU.add)
                _stt(nc, U2, sgv(5), 1.0, U2, ALU.mult, ALU.add)
                # A = (U1+U2)/3 + sig6 + sig7
                _stt(nc, A, U1, 1.0, U2, ALU.mult, ALU.add)
                _stt(nc, A, A, 1.0 / 3, sgv(6), ALU.mult, ALU.add)
                _stt(nc, A, sgv(7), 1.0, A, ALU.mult, ALU.add)
                # SUMP = (U1-U2)^2/9 + (U1/3-s6)^2 + (U1/3-s7)^2
                #      + (U2/3-s6)^2 + (U2/3-s7)^2 + (s6-s7)^2
                _stt(nc, DP, U1, 1.0, U2, ALU.mult, ALU.subtract)
                _stt(nc, SUMP, DP, 1.0 / 9, DP, ALU.mult, ALU.mult)
                for (uu, gg) in ((U1, 6), (U1, 7), (U2, 6), (U2, 7)):
                    _stt(nc, DP, uu, 1.0 / 3, sgv(gg), ALU.mult, ALU.subtract)
                    _stt(nc, DP, DP, 1.0, DP, ALU.mult, ALU.mult)
                    _stt(nc, SUMP, DP, 1.0, SUMP, ALU.mult, ALU.add)
                _stt(nc, DP, sgv(6), 1.0, sgv(7), ALU.mult, ALU.subtract)
                _stt(nc, DP, DP, 1.0, DP, ALU.mult, ALU.mult)
                _stt(nc, SUMP, DP, 1.0, SUMP, ALU.mult, ALU.add)
                # psi = SUMP / (12 * (A/4 + 1e-8)^2)
                nc.vector.tensor_scalar(
                    TA, A, 0.25, 1e-8, ALU.mult, ALU.add)
                _stt(nc, DEN, TA, 12.0, TA, ALU.mult, ALU.mult)
                with nc.allow_low_precision(reason="psi denom, 5e-4 rel ok"):
                    nc.vector.reciprocal(RD, DEN)
                _stt(nc, PSI, SUMP, 1.0, RD, ALU.mult, ALU.mult)
                # psi accumulate per seg (accum_out must be f32)
                for i in range(nb):
                    s = s0 + i
                    nc.vector.tensor_scalar(
                        pdum, PSI[:, i].rearrange("p i j -> p (i j)"),
                        1.0, None, ALU.mult, ALU.add,
                        accum_out=psi_acc[:, s:s + 1])
            wp_ctx.__exit__(None, None, None)

            # ===================== entropy phase ==========================
            ep_ctx = tc.tile_pool(name="ent", bufs=1)
            ep = ep_ctx.__enter__()
            ent_all = ep.tile([1, NPAT], F32)
            ps_ctx = tc.tile_pool(name="psum", bufs=2, space="PSUM")
            ps = ps_ctx.__enter__()
            for chk in range(n_chunks):
                n0 = chk * ch_pat
                # one-hot builds (DVE 4x): U*[q, u, j], V*[q, v, j]
                ng = ch_pat // 8
                UA = ep.tile([P, ng, 16, 8], F16, tag="UA", name="UA")
                UB = ep.tile([P, ng, 16, 8], F16, tag="UB", name="UB")
                VA = ep.tile([P, ng, 16, 8], F16, tag="VA", name="VA")
                VB = ep.tile([P, ng, 16, 8], F16, tag="VB", name="VB")
                hi_a = xphi_f[0][:, n0:n0 + ch_pat].rearrange(
                    "q (g r) -> q g r", r=8)
                hi_b = xphi_f[1][:, n0:n0 + ch_pat].rearrange(
                    "q (g r) -> q g r", r=8)
                lo_a = xplo_f[0][:, n0:n0 + ch_pat].rearrange(
                    "q (g r) -> q g r", r=8)
                lo_b = xplo_f[1][:, n0:n0 + ch_pat].rearrange(
                    "q (g r) -> q g r", r=8)
                for u in range(16):
                    nc.vector.tensor_scalar(
                        UA[:, :, u, :], hi_a, 16.0 * u, None, ALU.is_equal)
                    nc.vector.tensor_scalar(
                        UB[:, :, u, :], hi_b, 16.0 * u, None, ALU.is_equal)
                    nc.vector.tensor_scalar(
                        VA[:, :, u, :], lo_a, float(u), None, ALU.is_equal)
                    nc.vector.tensor_scalar(
                        VB[:, :, u, :], lo_b, float(u), None, ALU.is_equal)
                for bt in range(n_batch):
                    # 8 groups of 8 patches -> 8 [128, 128] histogram blocks
                    Mh = ps.tile([P, 8, P], F32, tag="Mh", name="Mh")
                    j0 = bt * 64
                    for j in range(8):
                        g = bt * 8 + j
                        nc.tensor.matmul(
                            Mh[:, j, :],
                            lhsT=UA[:, g].rearrange("p u r -> p (u r)"),
                            rhs=VA[:, g].rearrange("p u r -> p (u r)"),
                            start=True, stop=False)
                        nc.tensor.matmul(
                            Mh[:, j, :],
                            lhsT=UB[:, g].rearrange("p u r -> p (u r)"),
                            rhs=VB[:, g].rearrange("p u r -> p (u r)"),
                            start=False, stop=True)
                    Mf = Mh.rearrange("p j n -> p (j n)")
                    # L = ln(c + 1e-30)  (c=0 contributes 0 after the mult)
                    LT = ep.tile([P, 8 * P], F16, tag="LT", name="LT")
                    CT = ep.tile([P, 8 * P], F16, tag="CT", name="CT")
                    ET = ep.tile([P, 8, 16, 8], F16, tag="ET", name="ET")
                    nc.scalar.activation(LT, Mf, ACTF.Ln, bias=bias_eps)
                    nc.scalar.activation(CT, Mf, ACTF.Copy)
                    # E' = c * ln(c)  (gpsimd; all-SBUF operands)
                    nc.gpsimd.tensor_tensor(
                        ET.rearrange("p a b c -> p (a b c)"), CT, LT,
                        ALU.mult)
                    # tree-reduce over v (16 -> 1), fp16 4x
                    T1 = ep.tile([P, 8, 8, 8], F16, tag="T1", name="T1")
                    T2 = ep.tile([P, 8, 4, 8], F16, tag="T2", name="T2")
                    T3 = ep.tile([P, 8, 2, 8], F16, tag="T3", name="T3")
                    EV = ep.tile([P, 8, 1, 8], F16, tag="EV", name="EV")
                    _stt(nc, T1, ET[:, :, 0:8, :], 1.0, ET[:, :, 8:16, :],
                         ALU.mult, ALU.add)
                    _stt(nc, T2, T1[:, :, 0:4, :], 1.0, T1[:, :, 4:8, :],
                         ALU.mult, ALU.add)
                    _stt(nc, T3, T2[:, :, 0:2, :], 1.0, T2[:, :, 2:4, :],
                         ALU.mult, ALU.add)
                    _stt(nc, EV, T3[:, :, 0:1, :], 1.0, T3[:, :, 1:2, :],
                         ALU.mult, ALU.add)
                    # Q2[t, (g, r)] = sum_u EV[(u, r-part), (g, r)]
                    Q2 = ps.tile([8, 64], F32, tag="Q2", name="Q2")
                    nc.tensor.matmul(
                        Q2, lhsT=bd, rhs=EV.rearrange("p a o b -> p (a o b)"),
                        start=True, stop=True)
                    # mask diagonal r==t, then column-sum via ones matmul
                    MK = ep.tile([8, 64], F32, tag="MK", name="MK")
                    _stt(nc, MK, Q2, 1.0, mask64, ALU.mult, ALU.mult)
                    O6 = ps.tile([1, 64], F32, tag="O6", name="O6")
                    nc.tensor.matmul(O6, lhsT=ones8, rhs=MK,
                                     start=True, stop=True)
                    # stash into ent_all
                    nc.vector.tensor_scalar(
                        ent_all[:, n0 + j0:n0 + j0 + 64], O6, 1.0, None,
                        ALU.mult)
            # ent_all [1, (s, p)] -> ent_pm [128, s]  (48 tiny DMAs)
            for s in range(SEGS):
                nc.sync.dma_start(
                    ent_pm[:, s:s + 1],
                    ent_all[0:1, s * P:(s + 1) * P])
            ps_ctx.__exit__(None, None, None)
            ep_ctx.__exit__(None, None, None)

            # ============== combine: richness = psi_m * E =================
            # E = log2(196) - ENT / (196 ln 2);  psi_m = psi_acc / 144
            nc.vector.tensor_scalar(
                ent_pm, ent_pm, -1.0 / (NPIX * LN2), float(math.log2(NPIX)),
                ALU.mult, ALU.add)
            if debug_out == "ent":
                nc.vector.tensor_scalar(rich, ent_pm, 1.0, None, ALU.mult)
            elif debug_out == "psi":
                nc.vector.tensor_scalar(
                    rich, psi_acc, 1.0 / (NWIN * NWIN), None, ALU.mult)
            else:
                _stt(nc, rich, psi_acc, 1.0 / (NWIN * NWIN), ent_pm,
                     ALU.mult, ALU.mult)
            nc.vector.tensor_add(tsum, rich3[:, :, 0], rich3[:, :, 1])
            nc.vector.tensor_add(tsum, tsum, rich3[:, :, 2])
            nc.vector.tensor_scalar(osb, tsum, 1.0 / C, None, ALU.mult)

            # ---- output DMAs ----
            for t in range(T_BLKS):
                b = t // (T_BLKS // B_CORE)
                hp0 = (t % (T_BLKS // B_CORE)) * 4
                nc.sync.dma_start(ov[b, hp0:hp0 + 4], osb[:, t:t + 1])

    nc.compile()
    return nc


@functools.lru_cache(maxsize=4)
def _build_cached(dct_flat: tuple, debug_out: str = "rich") -> bass.Bass:
    return _build(dct_flat, debug_out=debug_out)


def kernel(x, dct_matrix):
    x = np.ascontiguousarray(np.asarray(x, dtype=np.float32))
    Dm = np.asarray(dct_matrix, dtype=np.float32)
    assert x.shape == (B_FULL, C, H, W), x.shape
    import os
    nc = _build_cached(tuple(float(v) for v in Dm.flatten()),
                       os.environ.get("KDBG", "rich"))
    in_maps = [
        {"x": np.ascontiguousarray(x[i * B_CORE:(i + 1) * B_CORE])}
        for i in range(N_CORES)
    ]
    res = bass_utils.run_bass_kernel_spmd(
        nc, in_maps, core_ids=list(range(N_CORES)))
    out = np.concatenate([r["out"] for r in res.results], axis=0)
    return out.astype(np.float32)
